# revision 68
# baseline (speedup 1.0000x reference)
"""Multi-head attention (b=2, n=2048, d=1024, H=16 heads) on 8 TRN2 NeuronCores.

Sharding: core c = (b, g) with b = c // 4 (data parallel over batch) and
g = c % 4 (tensor parallel over head groups of 4 heads).  Each core computes
qkv projections for its 4 heads, full softmax attention for those heads, and
a partial output projection y_partial = A_heads @ w_out[g*256:(g+1)*256].
The host sums the 4 partials per batch and adds b_out.

Final pipeline (per core), evolved over ~15 traced iterations from a 258 us
baseline to ~210 us (clock-normalized; the part shows run-to-run DVFS swings
of up to 20%, so compare timings via min-EXP-slice duration = 1003 ns):
  - ScalarE exp is the binding engine (139.8 us busy, 128 activations of
    [128, 2x512]); everything is scheduled to keep that stream gap-free.
  - flat 128-step loop (8 blocks (c,pr) x 16 key-tiles t); per step emit
    scores(s+1) BEFORE pv(s) so the PE always has the next step's scores
    in flight while exp(s) runs (removes head-of-line stalls; engine
    queues execute in a STATIC order chosen by the tile scheduler's sim).
  - host passes xt/weights PRE-SWIZZLED so every load is 128 contiguous
    4-8KB descriptors (strided slices cost ~14ns/descriptor issue), each
    tensor split over 4 DMA queues (one queue moves only ~25-50 GB/s);
    xt arrives in four n-chunks so the first k/q projection starts ~6 us
    after DMA boot instead of waiting for the full 4 MB.
  - ~80 const matmuls warm the PE during the DMA window: the clock ramps
    only on full-width work and drops back after a few idle us; a cold
    128x128x512 matmul costs 585 ns vs 216 warm.
  - extra PE work (qkv JIT groups, output projection) is spread at <= 2
    matmuls per step; proj for chunk c runs a FULL block after its at-tile
    completes and each matmul is order-pinned behind the attention stream
    (the sim undercosts InstReciprocal ~5x, so unpinned proj gets
    statically scheduled too early and head-of-line blocks the PE).
  - softmax epilogue: PV accumulators staged PSUM->SBUF immediately
    (frees the PSUM slots the next block's PV start needs); the 1024
    denominators (PV ones-column) sit on ONE partition where
    InstReciprocal costs ~7ns/elem, so they are DMA-gathered into
    [128, 8], inverted in one ~0.3us op, and DMA-scattered back to
    partition 0 (partition_broadcast requires a partition-0 source) --
    pure idle-queue latency, consumer is 1.5 blocks away.  Normalize
    pieces run at t12/t14 of the next block so JIT kq staging copies and
    proj ysb drains sit ahead of them in the DVE queue.
  - y is written in bf16 (halves output traffic; error budget allows it).
  - tail (last exp -> end): the 8 chunk-3 proj units get dedicated PSUM
    slots; all ks=0 matmuls (at pr=0 normalized a block earlier) plus two
    deferred chunk-2 units run DURING the denominator chain as p-state
    keep-alive, then the ks=1 pass is piece-pipelined per 128 at-columns;
    drain casts alternate DVE/ScalarE.
Matmuls run in bf16 (fp32 PSUM accumulation); measured end-to-end relative
error 5.5e-3 vs the fp32 reference (gate 2e-2).
"""

import os
import sys

for _p in ("/opt/trn_rl_repo",):
    if _p not in sys.path and os.path.isdir(_p):
        sys.path.insert(0, _p)

import ml_dtypes
import numpy as np

import concourse.bass as bass
import concourse.mybir as mybir
import concourse.tile as tile
from concourse import bacc

P = 128
D = 1024          # model dim
N = 2048          # sequence length
HD = 64           # head dim
GH = 4            # heads per core
DG = GH * HD      # 256 projected cols per core
KD = D // P       # 8 k-tiles over model dim
NT = N // P       # 16 tiles over sequence
QC = 512          # n_q chunk size
NQC = N // QC     # 4 chunks
SCALE = HD ** -0.5

F32 = mybir.dt.float32
BF16 = mybir.dt.bfloat16

Exp = mybir.ActivationFunctionType.Exp


def build_nc():
    nc = bacc.Bacc("TRN2")

    # all inputs pre-swizzled on host to [128 partitions, ...contiguous]
    xt = nc.declare_dram_parameter("xt", [P, NQC, KD, QC], BF16, isOutput=False)
    wq = nc.declare_dram_parameter("wq", [P, KD, DG], BF16, isOutput=False)
    wk = nc.declare_dram_parameter("wk", [P, KD, DG], BF16, isOutput=False)
    wv = nc.declare_dram_parameter("wv", [P, KD, DG], BF16, isOutput=False)
    wo = nc.declare_dram_parameter("wo", [P, 2, D], BF16, isOutput=False)
    y = nc.declare_dram_parameter("y", [N, D], BF16, isOutput=True)

    y_r = y[:, :].rearrange("(o p) n -> p o n", p=P)      # [128, 16, 1024]

    with tile.TileContext(nc) as tc, nc.allow_low_precision("bf16 attention"):
        with (
            tc.tile_pool(name="wpool", bufs=1) as wpool,
            tc.tile_pool(name="qkvpool", bufs=1) as qkvpool,
            tc.tile_pool(name="attnpool", bufs=1) as attnpool,
            tc.tile_pool(name="xpool", bufs=1) as xpool,
            tc.tile_pool(name="work", bufs=4) as work,
            tc.tile_pool(name="epi", bufs=2) as epi,
            tc.tile_pool(name="outp", bufs=8) as outp,
            tc.tile_pool(name="ps_a", bufs=2, space="PSUM") as ps_a,
            tc.tile_pool(name="ps_st", bufs=2, space="PSUM") as ps_st,
            tc.tile_pool(name="ps_o", bufs=2, space="PSUM") as ps_o,
        ):
            # --- persistent SBUF tiles ---
            wk_sb = wpool.tile([P, KD, DG], BF16, tag="wk")
            wq_sb = wpool.tile([P, KD, DG], BF16, tag="wq")
            wv_sb = wpool.tile([P, KD, DG], BF16, tag="wv")
            wo_sb = wpool.tile([P, 2, D], BF16, tag="wo")
            xt_sb = xpool.tile([P, NQC, KD, QC], BF16, tag="xt")

            qt_sb = qkvpool.tile([P, 2, N], BF16, tag="qt")   # [256, 2048] qT
            kt_sb = qkvpool.tile([P, 2, N], BF16, tag="kt")   # [256, 2048] kT
            vg_sb = qkvpool.tile([P, NT, GH, 66], BF16, tag="vg")  # v + ones col
            at_sb = attnpool.tile([P, 2, N], BF16, tag="at")  # attn_outT [256, 2048]

            # --- DMA in consumption order, split by k-pairs so each tensor
            # spreads over 4 queues (a single queue moves ~25-50 GB/s) ---
            def dma4(dst, src):
                for k2 in range(0, KD, 2):
                    nc.sync.dma_start(dst[:, k2:k2 + 2], src[:, k2:k2 + 2])

            dma4(wq_sb, wq[:, :, :])
            dma4(wk_sb, wk[:, :, :])
            for k in range(KD):
                nc.sync.dma_start(xt_sb[:, 0, k], xt[:, 0, k])
            dma4(wv_sb, wv[:, :, :])
            dma4(xt_sb[:, 1], xt[:, 1])
            dma4(xt_sb[:, 2], xt[:, 2])
            dma4(xt_sb[:, 3], xt[:, 3])
            for h2 in range(2):
                nc.sync.dma_start(wo_sb[:, h2], wo[:, h2])

            nc.scalar.copy(
                vg_sb[:, :, :, HD:], nc.const_aps.tensor(1.0, (P, NT, GH, 2), F32)
            )

            # PE p-state warmup during the DMA window: the Tensor engine only
            # reaches max clock after ~3us of continuous execution, so burn
            # cheap rank-1 matmuls on const data while inputs stream in.
            # full-width ops: narrow (contract-1) warmups only ever reach the
            # mid p-state; the clock ramps on full-array work
            warm_w = nc.const_aps.tensor(1.0, (P, P), BF16)
            warm_ps = ps_st.tile([P, 2, QC], F32, tag="st", name="warm")
            for _ in range(78):
                nc.tensor.matmul(
                    warm_ps[:, 0, 0:P], warm_w, warm_w, start=True, stop=True
                )

            # ---------------- emitters ----------------
            def emit_kq_piece(which, w_sb, dst, m, c, ks, state):
                # resumable half of an 8-matmul k/q projection group
                if state.get("ps") is None:
                    state["ps"] = ps_a.tile(
                        [P, QC], F32, tag="a", name=f"{which}ps_{m}_{c}"
                    )
                ps = state["ps"]
                for k in ks:
                    nc.tensor.matmul(
                        ps[:],
                        w_sb[:, k, m * P:(m + 1) * P],
                        xt_sb[:, c, k, :],
                        start=(k == 0),
                        stop=(k == KD - 1),
                    )
                if ks[-1] == KD - 1:
                    nc.vector.tensor_copy(dst[:, m, c * QC:(c + 1) * QC], ps[:])

            def emit_kq_group(which, w_sb, dst, m, c):
                emit_kq_piece(which, w_sb, dst, m, c, list(range(KD)), {})

            def emit_v(t):
                c4, r4 = t // 4, t % 4
                ps = ps_a.tile([P, QC], F32, tag="a", name=f"vps_{t}")
                for k in range(KD):
                    nc.tensor.matmul(
                        ps[:, :DG],
                        xt_sb[:, c4, k, r4 * P:(r4 + 1) * P],
                        wv_sb[:, k, :],
                        start=(k == 0),
                        stop=(k == KD - 1),
                    )
                nc.vector.tensor_copy(
                    vg_sb[:, t, :, 0:HD],
                    ps[:, :DG].rearrange("p (h e) -> p h e", h=GH),
                )

            def emit_scores(c, pr, t):
                cs = slice(c * QC, (c + 1) * QC)
                ts_ = slice(t * P, (t + 1) * P)
                st = ps_st.tile([P, 2, QC], F32, tag="st", name=f"st_{c}_{pr}_{t}")
                for half in range(2):
                    hs = slice(half * HD, (half + 1) * HD)
                    nc.tensor.matmul(
                        st[:, half, :],
                        kt_sb[hs, pr, ts_],
                        qt_sb[hs, pr, cs],
                        start=True,
                        stop=True,
                    )
                return st

            def emit_exp(c, pr, t, st):
                e = work.tile([P, 2, QC], BF16, tag="exp", name=f"e_{c}_{pr}_{t}")
                nc.scalar.activation(e[:], st[:], Exp, scale=SCALE)
                return e

            def emit_pv(c, pr, t, e, o_ps):
                last = None
                for half in range(2):
                    h = 2 * pr + half
                    last = nc.tensor.matmul(
                        o_ps[half][:],
                        vg_sb[:, t, h, 0:HD + 1],
                        e[:, half, :],
                        start=(t == 0),
                        stop=(t == NT - 1),
                    )
                return last

            def alloc_o(c, pr):
                o_ps = []
                for half in range(2):
                    o_full = ps_o.tile(
                        [P, QC], F32, tag="o", name=f"o_{c}_{pr}_{half}"
                    )
                    o_ps.append(o_full[: HD + 1])
                return o_ps

            def emit_epi_stage(c, pr, o_ps):
                # stage both PV accumulators to one SBUF tile right after the
                # PV stop (frees the PSUM slots the next block's PV reuses),
                # then invert the denominators TRANSPOSED: the 1024 denoms
                # live on ONE partition, where InstReciprocal costs ~7ns/elem
                # serially and clogs the DVE queue (stalling the proj ysb
                # drains whose ps_a release gates the PE).  A DMA partition-
                # gather turns them into [128, 8], the reciprocal becomes one
                # ~0.3us op, and a DMA scatter lands the result back on
                # partition 0 for the broadcasts.  Pure idle-queue latency;
                # the consumer is 1.5 blocks away.
                o_sb = epi.tile([HD + 1, 2, QC], F32, tag="osb", name=f"osb_{c}_{pr}")
                dT = epi.tile([P, 8], F32, tag="dT", name=f"dT_{c}_{pr}")
                for half in range(2):
                    nc.vector.tensor_copy(o_sb[:, half, :], o_ps[half][:])
                    nc.sync.dma_start(
                        dT[HD * half:HD * (half + 1), :],
                        o_sb[HD:HD + 1, half, :].rearrange(
                            "o (b i) -> o b i", i=8
                        ),
                    )
                rcT = epi.tile([P, 8], F32, tag="rcT", name=f"rcT_{c}_{pr}")
                nc.vector.reciprocal(rcT[:], dT[:, :])
                rc = epi.tile([1, 2, QC], F32, tag="rc", name=f"rc_{c}_{pr}")
                nc.sync.dma_start(
                    rc[0:1, :, :].rearrange("o h (b i) -> o (h b) i", i=8),
                    rcT[:, :],
                )
                return o_sb, rc

            def emit_epi_norm(c, pr, o_sb, rc, s, pieces):
                # one piece of the normalization: A^T = o[:64]*(1/o[64]).
                # rc=None (tail): reciprocal directly on the denom row — the
                # DMA-transposed path costs ~4.4us of roundtrip latency that
                # is pure critical path when nothing else is in flight.
                rbs = epi.tile([HD, 2, QC], F32, tag="rbs", name=f"rbs_{c}_{pr}",
                               uniquify=True)
                w = QC // pieces
                ss = slice(s * w, (s + 1) * w)
                if rc is None:
                    rc = epi.tile([1, 2, QC], F32, tag="rc",
                                  name=f"rcd_{c}_{pr}", uniquify=True)
                    for half in range(2):
                        nc.vector.reciprocal(
                            rc[:, half, ss], o_sb[HD:HD + 1, half, ss]
                        )
                nc.gpsimd.partition_broadcast(
                    rbs[:, :, ss], rc[0:1, :, ss], channels=HD
                )
                # half 1 first: its at-write goes through an extra SBUF->SBUF
                # DMA hop (partition shift), so start that chain earliest
                stg = work.tile(
                    [HD, w], BF16, tag="stg", name=f"stg_{c}_{pr}_{s}"
                )
                nc.vector.tensor_mul(stg[:], o_sb[0:HD, 1, ss], rbs[:, 1, ss])
                nc.sync.dma_start(
                    at_sb[HD:P, pr, c * QC + s * w:c * QC + (s + 1) * w],
                    stg[:],
                )
                nc.vector.tensor_mul(
                    at_sb[0:HD, pr, c * QC + s * w:c * QC + (s + 1) * w],
                    o_sb[0:HD, 0, ss],
                    rbs[:, 0, ss],
                )

            def emit_proj_mm(m, nn, ks, state, gate=None):
                # one matmul of a projection unit; ks==1 finishes + drains
                if state.get("ps") is None:
                    state["ps"] = ps_a.tile(
                        [P, QC], F32, tag="a", name=f"yps_{m}_{nn}"
                    )
                ps = state["ps"]
                mm = nc.tensor.matmul(
                    ps[:],
                    at_sb[:, ks, m * P:(m + 1) * P],
                    wo_sb[:, ks, nn * QC:(nn + 1) * QC],
                    start=(ks == 0),
                    stop=(ks == 1),
                )
                if gate is not None:
                    # order-only pin behind the attention stream: the sim
                    # undercosts the DVE reciprocal chain feeding at_sb, and
                    # an optimistically-early proj in the static PE queue
                    # head-of-line blocks the scores on hardware
                    bass._add_dep_helper(mm.ins, gate.ins, sync=False,
                                         reason="defer proj")
                if ks == 1:
                    ysb = outp.tile([P, QC], BF16, tag="y", name=f"y_{m}_{nn}")
                    nc.vector.tensor_copy(ysb[:], ps[:])
                    nc.sync.dma_start(y_r[:, m, nn * QC:(nn + 1) * QC], ysb[:])

            def emit_proj_unit(m, nn):
                state = {}
                emit_proj_mm(m, nn, 0, state)
                emit_proj_mm(m, nn, 1, state)

            # ---------------- extras schedule ----------------
            # blocks in c-major order: B = 2c + pr
            extras = {s: [] for s in range(128)}

            def sched_kq(which, w_sb, dst, m, c, B, t0):
                state = {}
                extras[B * NT + t0].append(
                    lambda gate, st=state: emit_kq_piece(
                        which, w_sb, dst, m, c, [0, 1, 2, 3], st
                    )
                )
                extras[B * NT + t0 + 1].append(
                    lambda gate, st=state: emit_kq_piece(
                        which, w_sb, dst, m, c, [4, 5, 6, 7], st
                    )
                )

            # block 0 carries v plus the JIT kq work for itself and block 1.
            # v runs one step AHEAD of its PV consumer: in the congested
            # block-0 schedule the v staging copy otherwise lands exactly
            # when PV(t) needs it, exposing the DVE latency every step
            for t in range(NT - 1):
                extras[t].append(lambda gate, tt=t + 1: emit_v(tt))
            sched_kq("k", wk_sb, kt_sb, 0, 1, 0, 0)
            sched_kq("k", wk_sb, kt_sb, 0, 2, 0, 4)
            sched_kq("k", wk_sb, kt_sb, 0, 3, 0, 8)
            sched_kq("k", wk_sb, kt_sb, 1, 0, 0, 10)
            sched_kq("q", wq_sb, qt_sb, 1, 0, 0, 12)
            # block 1 (0,1): rest of kt m1, then qt m0 c1 for block 2
            sched_kq("k", wk_sb, kt_sb, 1, 1, 1, 0)
            sched_kq("k", wk_sb, kt_sb, 1, 2, 1, 4)
            sched_kq("k", wk_sb, kt_sb, 1, 3, 1, 8)
            sched_kq("q", wq_sb, qt_sb, 0, 1, 1, 12)
            # later blocks: one qt group each, two steps
            sched_kq("q", wq_sb, qt_sb, 1, 1, 2, 2)
            sched_kq("q", wq_sb, qt_sb, 0, 2, 3, 2)
            sched_kq("q", wq_sb, qt_sb, 1, 2, 4, 2)
            sched_kq("q", wq_sb, qt_sb, 0, 3, 5, 2)
            sched_kq("q", wq_sb, qt_sb, 1, 3, 6, 2)
            # projection for chunk c spreads over block 2c+3 (a full block
            # after the at-chunk is complete), steps 4..15, pinned
            for c in range(3):
                B = 2 * c + 3
                states = [{} for _ in range(8)]
                for i in range(16):
                    if c == 2 and i >= 12:
                        continue  # m=11 units run in the tail as PE fill
                    j, ks = i // 2, i % 2
                    step = B * NT + 4 + (11 * i) // 16
                    extras[step].append(
                        lambda gate, m=4 * c + j // 2, nn=j % 2, ks=ks,
                            st=states[j]: emit_proj_mm(m, nn, ks, st, gate)
                    )

            # ---------------- prefix + main pipeline ----------------
            # prefix k/q groups interleaved per k-slice: both consume xt
            # chunk-0 slices as they land, leaving only two matmuls (not a
            # whole group) after the last slice arrives
            kst, qst = {}, {}
            for k in range(KD):
                emit_kq_piece("k", wk_sb, kt_sb, 0, 0, [k], kst)
                emit_kq_piece("q", wq_sb, qt_sb, 0, 0, [k], qst)

            blocks = [(c, pr) for c in range(NQC) for pr in range(2)]
            sts = {}
            sts[0] = emit_scores(0, 0, 0)
            emit_v(0)
            o_ps = None
            last_pv = None
            pend_norm = None          # (c, pr, o_sb) of the previous block
            for s in range(128):
                B, t = s // NT, s % NT
                c, pr = blocks[B]
                if t == 0:
                    o_ps = alloc_o(c, pr)
                if s + 1 < 128:
                    Bn, tn = (s + 1) // NT, (s + 1) % NT
                    cn, prn = blocks[Bn]
                    sts[s + 1] = emit_scores(cn, prn, tn)
                e = emit_exp(c, pr, t, sts.pop(s))
                for fn in extras[s]:
                    fn(last_pv)
                # normalization pieces of the previous block's epilogue run
                # at t=12/14: late enough that this block's JIT kq staging
                # and proj ysb drains sit ahead of the (slow, sim-undercosted)
                # reciprocals in the DVE queue; the at-chunk consumer is a
                # full block later still
                if pend_norm is not None and t in (12, 14):
                    pc, ppr, po_sb, prc = pend_norm
                    emit_epi_norm(pc, ppr, po_sb, prc, 0 if t == 12 else 1, 2)
                    if t == 14:
                        pend_norm = None
                last_pv = emit_pv(c, pr, t, e, o_ps)
                if t == NT - 1 and B < 7:
                    o_sb, rc = emit_epi_stage(c, pr, o_ps)
                    pend_norm = (c, pr, o_sb, rc)

            # ---------------- tail: last epilogue + chunk-3 projection ----
            # at[:, 0, :] (pr=0) was normalized a block ago, so all eight
            # ks=0 matmuls can run DURING the last norm chain — each proj
            # unit gets its own PSUM slot (the st/o/a pools are idle now).
            # A naive ks0/ks1-interleaved emission head-of-line blocks the
            # PE queue on the first ks=1 matmul for the whole norm latency.
            o_sb, rc_t = emit_epi_stage(3, 1, o_ps)
            # deferred chunk-2 units double as p-state keep-alive while the
            # denominator chain runs
            emit_proj_unit(11, 0)
            emit_proj_unit(11, 1)
            st_a = ps_st.tile([P, 2, QC], F32, tag="st", name="tps_a")
            st_b = ps_st.tile([P, 2, QC], F32, tag="st", name="tps_b")
            o_a = ps_o.tile([P, QC], F32, tag="o", name="tps_c")
            o_b = ps_o.tile([P, QC], F32, tag="o", name="tps_d")
            a_a = ps_a.tile([P, QC], F32, tag="a", name="tps_e")
            a_b = ps_a.tile([P, QC], F32, tag="a", name="tps_f")
            slots = [st_a[:, 0, :], st_a[:, 1, :], st_b[:, 0, :], st_b[:, 1, :],
                     o_a, o_b, a_a, a_b]
            units = [(12 + j // 2, j % 2) for j in range(8)]
            for j, (m, nn) in enumerate(units):
                nc.tensor.matmul(
                    slots[j],
                    at_sb[:, 0, m * P:(m + 1) * P],
                    wo_sb[:, 0, nn * QC:(nn + 1) * QC],
                    start=True,
                    stop=False,
                )
            # ks=1 pass piece-pipelined per m-tile: normalize 128 at-columns,
            # then immediately run the two matmuls that consume them
            rbs_t = epi.tile([HD, 2, QC], F32, tag="rbs", name="rbs_t")
            for p in range(4):
                ss = slice(p * P, (p + 1) * P)
                cs = slice(3 * QC + p * P, 3 * QC + (p + 1) * P)
                for half in range(2):
                    nc.gpsimd.partition_broadcast(
                        rbs_t[:, half, ss], rc_t[0:1, half, ss], channels=HD
                    )
                stg = work.tile([HD, P], BF16, tag="stg", name=f"stg_t_{p}")
                nc.vector.tensor_mul(stg[:], o_sb[0:HD, 1, ss], rbs_t[:, 1, ss])
                nc.sync.dma_start(at_sb[HD:P, 1, cs], stg[:])
                nc.vector.tensor_mul(
                    at_sb[0:HD, 1, cs], o_sb[0:HD, 0, ss], rbs_t[:, 0, ss]
                )
                for nn in range(2):
                    j = 2 * p + nn
                    m = 12 + p
                    nc.tensor.matmul(
                        slots[j],
                        at_sb[:, 1, m * P:(m + 1) * P],
                        wo_sb[:, 1, nn * QC:(nn + 1) * QC],
                        start=False,
                        stop=True,
                    )
                    # split the drain casts between DVE and the (now idle)
                    # Scalar engine so they don't serialize
                    ysb = outp.tile([P, QC], BF16, tag="y", name=f"yt_{m}_{nn}")
                    if j % 2 == 0:
                        nc.vector.tensor_copy(ysb[:], slots[j])
                    else:
                        nc.scalar.copy(ysb[:], slots[j])
                    nc.sync.dma_start(y_r[:, m, nn * QC:(nn + 1) * QC], ysb[:])

    nc.finalize()
    return nc


_NC = None


def _get_nc():
    global _NC
    if _NC is None:
        _NC = build_nc()
    return _NC


def _swiz_w(w):
    # [1024, cols] -> [128, 8, cols]: partition-contiguous for 1-desc rows
    return np.ascontiguousarray(
        w.reshape(KD, P, w.shape[1]).transpose(1, 0, 2)
    )


def _in_maps(x, w_qkv, w_out):
    bf = ml_dtypes.bfloat16
    x = np.asarray(x, dtype=np.float32)
    w_qkv = np.asarray(w_qkv, dtype=np.float32)
    w_out = np.asarray(w_out, dtype=np.float32)
    # xt[p, chunk, k, n'] = x[b].T[k*128+p, chunk*512+n']
    xts = []
    for b in range(2):
        xtb = x[b].T.reshape(KD, P, NQC, QC).transpose(1, 2, 0, 3)
        xts.append(np.ascontiguousarray(xtb).astype(bf))
    wq_g = [_swiz_w(w_qkv[:, 0 * D + g * DG:0 * D + (g + 1) * DG]).astype(bf) for g in range(4)]
    wk_g = [_swiz_w(w_qkv[:, 1 * D + g * DG:1 * D + (g + 1) * DG]).astype(bf) for g in range(4)]
    wv_g = [_swiz_w(w_qkv[:, 2 * D + g * DG:2 * D + (g + 1) * DG]).astype(bf) for g in range(4)]
    wo_g = [
        np.ascontiguousarray(
            w_out[g * DG:(g + 1) * DG, :].reshape(2, P, D).transpose(1, 0, 2)
        ).astype(bf)
        for g in range(4)
    ]
    maps = []
    for c in range(8):
        b, g = c // 4, c % 4
        maps.append({
            "xt": xts[b],
            "wq": wq_g[g],
            "wk": wk_g[g],
            "wv": wv_g[g],
            "wo": wo_g[g],
        })
    return maps


LAST_RESULT = None


def kernel(x, w_qkv, w_out, b_out):
    from concourse.bass_utils import run_bass_kernel_spmd

    nc = _get_nc()
    maps = _in_maps(x, w_qkv, w_out)
    res = run_bass_kernel_spmd(nc, maps, list(range(8)))
    global LAST_RESULT
    LAST_RESULT = res
    out = np.zeros((2, N, D), dtype=np.float32)
    for c in range(8):
        out[c // 4] += np.asarray(res.results[c]["y"], dtype=np.float32)
    out += np.asarray(b_out, dtype=np.float32)[None, None, :]
    return out
